# revision 1
# baseline (speedup 1.0000x reference)
"""CMHSA Trainium2 kernel v3 (nn_CMHSA_56487409877161).

v3 structure per core (4 batches):
  startconv fwd (splain/spos, bf16) and transposed (sT2 with ones col),
  qsum via DVE free-axis reduce on splain.
  Per head: ET matmuls (softmax axis on partitions), exp on ACT,
  P^2 on DVE; G+r fused in one stream (lhsT = [sT_h | ones], 65 rows out),
  ssq via eyes-accumulated P^2 stream ([8, N] psum rows 96-103).
  Stats batched on 8 partitions. Phase B bf16: H = WlT' G (bf16 rhs),
  c broadcast via pair select-matmul (bf16), F tail on DVE.
"""

import numpy as np

import concourse.bass as bass
import concourse.mybir as mybir
import concourse.tile as tile
from concourse import bacc, bass_utils

B, C, N = 32, 512, 1024
HEADS, DH = 8, 64
NCORES = 8
BPC = B // NCORES
EPS = 1e-5
SCALE = (C / 4.0) ** 0.5
SQ = float(np.sqrt(SCALE))
EBIAS = 45.0
MU = 1.0 / N
BUILD_SALT = 107  # bumps HLO signature to defeat stale executable caches

F32 = mybir.dt.float32
BF16 = mybir.dt.bfloat16
AF = mybir.ActivationFunctionType
ALU = mybir.AluOpType

MMDT = BF16


def build_program():
    nc = bacc.Bacc("TRN2", target_bir_lowering=False)
    dt = F32
    pdt = BF16
    xin = nc.dram_tensor("xin", [BPC, C, N], MMDT, kind="ExternalInput").ap()
    xrd = nc.dram_tensor("xrd", [BPC, C, N], F32, kind="ExternalInput").ap()
    wco = nc.dram_tensor("wco", [C, C], MMDT, kind="ExternalInput").ap()
    posd = nc.dram_tensor("posd", [C, N], dt, kind="ExternalInput").ap()
    bc128 = nc.dram_tensor("bc128", [128, 4], dt, kind="ExternalInput").ap()
    eyed = nc.dram_tensor("eyed", [128, 128], MMDT, kind="ExternalInput").ap()
    wl = nc.dram_tensor("wl", [128, DH], F32, kind="ExternalInput").ap()
    wlt = nc.dram_tensor("wlt", [DH, DH], MMDT, kind="ExternalInput").ap()
    blin2 = nc.dram_tensor("blin2", [128, 1], dt, kind="ExternalInput").ap()
    eyesd = nc.dram_tensor("eyesd", [128, HEADS, HEADS], pdt, kind="ExternalInput").ap()
    sel2d = nc.dram_tensor("sel2d", [HEADS, 4, 128], pdt, kind="ExternalInput").ap()
    oner = nc.dram_tensor("oner", [1, 128], F32, kind="ExternalInput").ap()
    eyef8 = nc.dram_tensor("eyef8", [HEADS, HEADS], F32, kind="ExternalInput").ap()
    cbias = nc.dram_tensor("cbias", [128, 3 + BUILD_SALT], dt, kind="ExternalInput").ap()
    outd = nc.dram_tensor("outd", [BPC, C, N], dt, kind="ExternalOutput").ap()

    act = nc.scalar
    vec = nc.vector
    pe = nc.tensor

    with tile.TileContext(nc) as tc:
        with (
            tc.tile_pool(name="consts", bufs=1) as consts,
            tc.tile_pool(name="xpool", bufs=1) as xpool,
            tc.tile_pool(name="spool", bufs=1) as spool,
            tc.tile_pool(name="stpool", bufs=1) as stpool,
            tc.tile_pool(name="gpool", bufs=1) as gpool,
            tc.tile_pool(name="ppool", bufs=6) as ppool,
            tc.tile_pool(name="p2pool", bufs=4) as p2pool,
            tc.tile_pool(name="tails", bufs=2) as tails,
            tc.tile_pool(name="stats", bufs=1) as stats,
            tc.tile_pool(name="psum", bufs=1, space="PSUM") as psum,
        ):
            wco_sb = consts.tile([128, 4, C], MMDT)
            nc.sync.dma_start(wco_sb[:], wco.rearrange("(cc p) o -> p cc o", p=128))
            pos_sb = consts.tile([128, 4, N], dt)
            nc.sync.dma_start(pos_sb[:], posd.rearrange("(cc p) n -> p cc n", p=128))
            bc128_sb = consts.tile([128, 4], dt)
            nc.sync.dma_start(bc128_sb[:], bc128)
            eyed_sb = consts.tile([128, 128], MMDT)
            nc.sync.dma_start(eyed_sb[:], eyed)
            wl_sb = consts.tile([128, DH], F32)
            nc.sync.dma_start(wl_sb[:], wl)
            wlt_sb = consts.tile([DH, DH], MMDT)
            nc.sync.dma_start(wlt_sb[:], wlt)
            blin2_sb = consts.tile([128, 1], dt)
            nc.sync.dma_start(blin2_sb[:], blin2)
            eyes_sb = consts.tile([128, HEADS, HEADS], pdt)
            nc.sync.dma_start(eyes_sb[:], eyesd)
            sel2_sb = consts.tile([HEADS, 4, 128], pdt)
            nc.sync.dma_start(sel2_sb[:], sel2d)
            oner_sb = consts.tile([1, 128], F32)
            nc.sync.dma_start(oner_sb[:], oner)
            eyef8_sb = consts.tile([HEADS, HEADS], F32)
            nc.sync.dma_start(eyef8_sb[:], eyef8)
            cb_sb = consts.tile([128, 3], dt)
            nc.sync.dma_start(cb_sb[:], cbias[:, 0:3])

            # sT2 buffers: [128 n, 8 nt, 8 h, 65] with ones in col 64.
            # Two buffers alternated across batches for pipelining.
            st2_bufs = []
            for i in range(2):
                t = stpool.tile([128, 8, HEADS, DH + 1], MMDT, name=f"st2_{i}")
                vec.memset(t[:, :, :, DH : DH + 1], 1.0)
                st2_bufs.append(t)

            B_state = {}

            def make_batch(b):
                st = {}

                def prefront():
                    x_sb = xpool.tile([128, 4, N], MMDT, tag="x", name=f"x_{b}")
                    nc.sync.dma_start(
                        x_sb[:], xin[b].rearrange("(cc p) n -> p cc n", p=128)
                    )
                    splain = spool.tile([128, 4, N], MMDT, tag="splain", name=f"splain_{b}")
                    spos = spool.tile([128, 4, N], MMDT, tag="spos", name=f"spos_{b}")
                    sT2 = st2_bufs[b % 2]

                    def emit_sconv(pc):
                        s_ps = psum.tile([128, N], dt, tag="et", bufs=2, name=f"sps_{b}_{pc}")
                        for cc in range(4):
                            for half in range(2):
                                pe.matmul(
                                    s_ps[:, half * 512 : half * 512 + 512],
                                    lhsT=wco_sb[:, cc, 128 * pc : 128 * pc + 128],
                                    rhs=x_sb[:, cc, half * 512 : half * 512 + 512],
                                    start=(cc == 0),
                                    stop=(cc == 3),
                                )
                        act.activation(
                            splain[:, pc, :], s_ps[:], AF.Identity,
                            bias=bc128_sb[:, pc : pc + 1], scale=1.0,
                        )
                        vec.scalar_tensor_tensor(
                            out=spos[:, pc, :], in0=s_ps[:],
                            scalar=bc128_sb[:, pc : pc + 1],
                            in1=pos_sb[:, pc, :], op0=ALU.add, op1=ALU.add,
                        )

                    def emit_trans(pc):
                        st_ps = psum.tile([128, N], MMDT, tag="et", bufs=2, name=f"stps_{b}_{pc}")
                        for nt in range(8):
                            pe.transpose(
                                st_ps[:, 128 * nt : 128 * nt + 128],
                                in_=splain[:, pc, 128 * nt : 128 * nt + 128],
                                identity=eyed_sb[:],
                            )
                        stv = st_ps.rearrange("p (nt two d) -> p nt two d", nt=8, two=2)
                        act.activation(sT2[:, :, 2 * pc, 0:DH], stv[:, :, 0, :], AF.Copy)
                        act.activation(sT2[:, :, 2 * pc + 1, 0:DH], stv[:, :, 1, :], AF.Copy)

                    st.update(splain=splain, spos=spos, sT2=sT2,
                              emit_sconv=emit_sconv, emit_trans=emit_trans)
                    emit_sconv(0)
                    emit_sconv(1)
                    emit_trans(0)

                def rest():
                    splain, spos, sT2 = st["splain"], st["spos"], st["sT2"]
                    emit_sconv, emit_trans = st["emit_sconv"], st["emit_trans"]

                    qs_col = stats.tile([128, 4], dt, tag="qs_col", name=f"qscol_{b}")

                    def emit_qred(pc):
                        vec.reduce_sum(
                            qs_col[:, pc : pc + 1],
                            splain[:, pc : pc + 1, :],
                            axis=mybir.AxisListType.X,
                        )

                    def emit_wq():
                        qs_dmat = stats.tile([DH, HEADS], F32, tag="qsd", name=f"qsd_{b}")
                        for h in range(HEADS):
                            nc.sync.dma_start(
                                qs_dmat[:, h : h + 1],
                                qs_col[(h % 2) * 64 : (h % 2) * 64 + DH, h // 2 : h // 2 + 1],
                            )
                        wq_ps = psum.tile([DH, HEADS], dt, tag="gr", bufs=1, name=f"wqps_{b}")
                        pe.matmul(
                            wq_ps[:], lhsT=wl_sb[0:DH, :], rhs=qs_dmat[:],
                            start=True, stop=True,
                        )
                        wq_sb = stats.tile([DH, HEADS], dt, tag="wq", name=f"wq_{b}")
                        vec.tensor_copy(wq_sb[:], wq_ps[:])
                        return wq_sb

                    hooks = {
                        (0, 4): lambda: emit_trans(1),
                        (1, 0): lambda: emit_sconv(2),
                        (1, 4): lambda: emit_trans(2),
                        (2, 0): lambda: emit_sconv(3),
                        (2, 4): lambda: emit_trans(3),
                        (3, 0): lambda: emit_qred(0),
                        (3, 4): lambda: emit_qred(1),
                        (4, 0): lambda: emit_qred(2),
                        (4, 4): lambda: emit_qred(3),
                    }
                    wq_box = []
                    hooks[(6, 0)] = lambda: wq_box.append(emit_wq())

                    # ---------- per-head maps
                    ssq_ps = psum.tile([104, N], dt, tag="ssq", bufs=1, name=f"ssqps_{b}")
                    r_b = stats.tile([HEADS, N], dt, tag="r_b", name=f"rb_{b}")
                    g_tiles = {}
                    gr_tiles = {}
                    pq = []

                    def emit_gr(h_, sc_, p_sb_, p2_sb_):
                        gr_ = gr_tiles[h_]
                        for half in range(2):
                            sl = slice(half * 512, half * 512 + 512)
                            pe.matmul(
                                gr_[:, sl],
                                lhsT=sT2[:, sc_, h_, :],
                                rhs=p_sb_[:, sl],
                                start=(sc_ == 0),
                                stop=(sc_ == 7),
                            )
                        for half in range(2):
                            sl = slice(half * 512, half * 512 + 512)
                            pe.matmul(
                                ssq_ps[96:104, sl],
                                lhsT=eyes_sb[:, h_, :],
                                rhs=p2_sb_[:, sl],
                                start=(h_ == 0 and sc_ == 0),
                                stop=(h_ == 7 and sc_ == 7),
                                tile_position=(0, 96),
                            )

                    def evac_head(h_):
                        gr_ = gr_tiles[h_]
                        g_sb = gpool.tile([DH, N], MMDT, tag="g", bufs=9, name=f"g_{b}_{h_}")
                        g_tiles[h_] = g_sb
                        act.activation(g_sb[:], gr_[0:DH, :], AF.Copy)
                        r_row = stats.tile([1, N], dt, tag="r_row", bufs=2, name=f"rrow_{b}_{h_}")
                        vec.tensor_copy(r_row[:], gr_[DH : DH + 1, :])
                        nc.sync.dma_start(r_b[h_ : h_ + 1, :], r_row[:])

                    for h in range(HEADS):
                        prow = (h % 2) * 64
                        pcix = h // 2
                        gr_tiles[h] = psum.tile(
                            [DH + 1, N], dt, tag="gr", bufs=1, name=f"grps_{b}_{h}"
                        )
                        for sc in range(8):
                            et_ps = psum.tile([128, N], dt, tag="et", bufs=2, name=f"et_{b}_{h}_{sc}")
                            for half in range(2):
                                pe.matmul(
                                    et_ps[:, half * 512 : half * 512 + 512],
                                    lhsT=splain[prow : prow + 64, pcix, 128 * sc : 128 * sc + 128],
                                    rhs=spos[prow : prow + 64, pcix, half * 512 : half * 512 + 512],
                                    start=True,
                                    stop=True,
                                )
                            p_sb = ppool.tile([128, N], pdt, tag="p", name=f"p_{b}_{h}_{sc}")
                            act.activation(p_sb[:], et_ps[:], AF.Exp, bias=cb_sb[:, 0:1], scale=1.0)
                            p2_sb = p2pool.tile([128, N], pdt, tag="p2", name=f"p2_{b}_{h}_{sc}")
                            vec.tensor_tensor(p2_sb[:], p_sb[:], p_sb[:], ALU.mult)
                            pq.append((h, sc, p_sb, p2_sb))
                            hk = hooks.pop((h, sc), None)
                            if hk is not None:
                                hk()
                            if len(pq) > 1:
                                emit_gr(*pq.pop(0))
                                if sc == 0 and h > 0:
                                    evac_head(h - 1)
                    for item in pq:
                        emit_gr(*item)
                    evac_head(HEADS - 1)

                    # ---------- tail
                    xres_tiles = {}
                    for h in range(HEADS):
                        xres = tails.tile([DH, N], MMDT, tag="xres", bufs=9, name=f"xres_{b}_{h}")
                        nc.sync.dma_start(xres[:], xin[b, DH * h : DH * h + DH, :])
                        xres_tiles[h] = xres
                    # stats chain first on ACT/DVE; H matmuls cover it on the PE
                    ssq_sb = stats.tile([HEADS, N], dt, tag="ssq_sb", name=f"ssqsb_{b}")
                    vec.tensor_copy(ssq_sb[:], ssq_ps[96:104, :])
                    rinv = stats.tile([HEADS, N], dt, tag="rinv", name=f"rinv_{b}")
                    vec.reciprocal_approx_fast(rinv[:], r_b[:])
                    rinvsq = stats.tile([HEADS, N], dt, tag="rinvsq", name=f"rinvsq_{b}")
                    vec.tensor_tensor(rinvsq[:], rinv[:], rinv[:], ALU.mult)
                    ttr_scr = stats.tile([HEADS, N], dt, tag="ttr", name=f"ttr_{b}")
                    vec.tensor_tensor(ttr_scr[:], ssq_sb[:], rinvsq[:], ALU.mult)
                    s2 = stats.tile([HEADS, 1], dt, tag="s2", name=f"s2_{b}")
                    vec.reduce_sum(s2[:], ttr_scr[:], axis=mybir.AxisListType.X)
                    var = stats.tile([HEADS, 1], dt, tag="var", name=f"var_{b}")
                    vec.tensor_scalar(
                        out=var[:], in0=s2[:], scalar1=1.0 / (float(N) * float(N)),
                        scalar2=-MU * MU, op0=ALU.mult, op1=ALU.add,
                    )
                    lnv = stats.tile([HEADS, 1], dt, tag="lnv", name=f"lnv_{b}")
                    act.activation(lnv[:], var[:], AF.Ln, bias=cb_sb[0:HEADS, 1:2], scale=1.0)
                    istd = stats.tile([HEADS, 1], dt, tag="istd", name=f"istd_{b}")
                    act.activation(istd[:], lnv[:], AF.Exp, bias=cb_sb[0:HEADS, 2:3], scale=-0.5)
                    c_b = stats.tile([HEADS, N], dt, tag="c_b", name=f"cb_{b}")
                    vec.tensor_scalar(
                        out=c_b[:], in0=rinv[:], scalar1=istd[:], scalar2=None, op0=ALU.mult
                    )
                    c_bf = stats.tile([HEADS, N], MMDT, tag="c_bf", name=f"cbf_{b}")
                    vec.tensor_copy(c_bf[:], c_b[:])
                    h_sbs = {}
                    for h in range(HEADS):
                        h_ps = psum.tile([DH, N], dt, tag="et", bufs=2, name=f"hps_{b}_{h}")
                        for half in range(2):
                            sl = slice(half * 512, half * 512 + 512)
                            pe.matmul(
                                h_ps[:, sl], lhsT=wlt_sb[:],
                                rhs=g_tiles[h][:, sl],
                                start=True, stop=True,
                            )
                        h_sb = tails.tile([DH, N], MMDT, tag="h_sb", bufs=9, name=f"hsb_{b}_{h}")
                        act.activation(h_sb[:], h_ps[:], AF.Copy)
                        h_sbs[h] = h_sb

                    # prefront of the next batch: PE work that overlaps the
                    # stats chain above
                    if b + 1 < BPC:
                        B_state[b + 1] = make_batch(b + 1)
                        B_state[b + 1]["prefront"]()

                    it_ps = psum.tile([1, HEADS], dt, tag="gr", bufs=1, name=f"itps_{b}")
                    pe.transpose(it_ps[:], in_=istd[:], identity=eyef8_sb[:])
                    istd_t = stats.tile([1, HEADS], dt, tag="istd_t", name=f"istdt_{b}")
                    vec.tensor_copy(istd_t[:], it_ps[:])
                    ibc_ps = psum.tile([DH, HEADS], dt, tag="gr", bufs=1, name=f"ibcps_{b}")
                    pe.matmul(ibc_ps[:], lhsT=oner_sb[0:1, 0:DH], rhs=istd_t[:], start=True, stop=True)
                    istd_bc = stats.tile([DH, HEADS], dt, tag="istd_bc", name=f"istdbc_{b}")
                    act.activation(istd_bc[:], ibc_ps[:], AF.Copy)
                    beta_t = stats.tile([DH, HEADS], dt, tag="beta_t", name=f"betat_{b}")
                    vec.scalar_tensor_tensor(
                        out=beta_t[:], in0=wq_box[0][:], scalar=-MU, in1=istd_bc[:],
                        op0=ALU.mult, op1=ALU.mult,
                    )
                    beta_sb = stats.tile([DH, HEADS], dt, tag="beta", name=f"beta_{b}")
                    vec.tensor_scalar(
                        out=beta_sb[:], in0=beta_t[:], scalar1=blin2_sb[0:DH, :], scalar2=None,
                        op0=ALU.add,
                    )

                    # ---------- Phase B
                    for pair in range(4):
                        cbc_ps = psum.tile([128, N], dt, tag="ssq", bufs=1, name=f"cbc_{b}_{pair}")
                        for half in range(2):
                            sl = slice(half * 512, half * 512 + 512)
                            pe.matmul(
                                cbc_ps[:, sl], lhsT=sel2_sb[:, pair, :],
                                rhs=c_bf[:, sl],
                                start=True, stop=True,
                            )
                        for sub in range(2):
                            h = 2 * pair + sub
                            prow = sub * 64
                            xres = xres_tiles[h]
                            t1 = tails.tile([DH, N], dt, tag="t1", name=f"t1_{b}_{h}")
                            vec.tensor_tensor(
                                t1[:], cbc_ps[prow : prow + DH, :], h_sbs[h][:], ALU.mult
                            )
                            f_sb = tails.tile([DH, N], dt, tag="f", name=f"f_{b}_{h}")
                            vec.scalar_tensor_tensor(
                                out=f_sb[:], in0=t1[:], scalar=beta_sb[:, h : h + 1],
                                in1=xres[:], op0=ALU.add, op1=ALU.add,
                            )
                            nc.sync.dma_start(outd[b, DH * h : DH * h + DH, :], f_sb[:])

                st["prefront"] = prefront
                st["rest"] = rest
                return st

            B_state[0] = make_batch(0)
            B_state[0]["prefront"]()
            for b in range(BPC):
                B_state[b]["rest"]()

    nc.compile()
    return nc


def host_inputs(x, W_start, b_start, rel_h, rel_w, W_lin, b_lin):
    x = np.asarray(x, np.float32)
    W_start = np.asarray(W_start, np.float32)
    b_start = np.asarray(b_start, np.float32)
    pos = (np.asarray(rel_h, np.float32) + np.asarray(rel_w, np.float32)).reshape(
        HEADS, DH, N
    )
    W_lin = np.asarray(W_lin, np.float32)
    b_lin = np.asarray(b_lin, np.float32)
    import ml_dtypes

    bf = ml_dtypes.bfloat16
    sel2 = np.zeros((HEADS, 4, 128), np.float32)
    for p in range(4):
        sel2[2 * p, p, 0:64] = 1.0
        sel2[2 * p + 1, p, 64:128] = 1.0
    consts = {
        "wco": np.ascontiguousarray((W_start.T / SQ).astype(bf)),
        "posd": np.ascontiguousarray((pos * SQ).reshape(C, N).astype(np.float32)),
        "bc128": np.ascontiguousarray(
            (b_start / SQ).reshape(4, 128).T.astype(np.float32)
        ),
        "wl": np.ascontiguousarray(
            np.concatenate([(W_lin * SQ).T, (W_lin * SQ).T], axis=0).astype(np.float32)
        ),
        "wlt": np.ascontiguousarray((W_lin * SQ).T.astype(bf)),
        "blin2": np.ascontiguousarray(np.tile(b_lin, 2)[:, None].astype(np.float32)),
        "eyesd": np.ascontiguousarray(
            np.broadcast_to(np.eye(HEADS, dtype=np.float32), (128, HEADS, HEADS))
        ).astype(bf),
        "eyed": np.ascontiguousarray(np.eye(128, dtype=np.float32).astype(bf)),
        "sel2d": np.ascontiguousarray(sel2.astype(bf)),
        "oner": np.ones((1, 128), np.float32),
        "eyef8": np.ascontiguousarray(np.eye(HEADS, dtype=np.float32)),
        "cbias": np.ascontiguousarray(
            np.broadcast_to(
                np.array([-EBIAS, EPS, 0.0] + [0.0] * BUILD_SALT, np.float32),
                (128, 3 + BUILD_SALT),
            )
        ),
    }
    xr = x.reshape(B, C, N)
    in_maps = []
    for c in range(NCORES):
        m = dict(consts)
        m["xin"] = np.ascontiguousarray(xr[c * BPC : (c + 1) * BPC].astype(bf))
        m["xrd"] = np.ascontiguousarray(xr[c * BPC : (c + 1) * BPC])
        in_maps.append(m)
    return in_maps


_PROG = None


def kernel(**inputs):
    global _PROG
    if _PROG is None:
        _PROG = build_program()
    in_maps = host_inputs(**inputs)
    res = bass_utils.run_bass_kernel_spmd(_PROG, in_maps, core_ids=list(range(NCORES)))
    out = np.concatenate([r["outd"] for r in res.results], axis=0)
    return out.reshape(B, C, 32, 32)

